# revision 1
# baseline (speedup 1.0000x reference)
"""MoE-LoRA Linear kernel for 8x Trainium2 NeuronCores.

Math: out = x @ W^T + bias + sum_e gate[e] * (x @ A_e^T) @ B_e^T
  x [4,2048,4096], W [4096,4096], A [8,8,4096], B [8,4096,8].
  gate = softmax(router(expert_embed)) top-2 masked * scaling (per-task
  scalars, computed on host: 8 numbers).

Device strategy (data-parallel over the 8192 tokens, 1024/core):
  - host pre-transposes x and W so the contraction dim d lands on SBUF
    partitions: xT [4096,1024] per core, WT [4096,4096] replicated.
  - per core: x^T resident in SBUF (16.8 MB); W streamed once; out =
    (x^T).T @ W^T accumulated in PSUM over 32 d-tiles (fp32r matmuls,
    N=512 -> full-rate PE).
  - LoRA: hT = A_all @ x^T ([64,1024]) computed on device, then one
    extra accumulating bf16 matmul per PSUM tile (hT against the
    host-side gate-scaled B) adds the LoRA term; the fp32 bias rides on
    the DVE eviction add. o-tile 0 gets its LoRA post-hoc so hT (which
    needs all of x) never stalls the PE behind the x load.
"""

import numpy as np

B_, S, D = 4, 2048, 4096
O = 4096
N_CORES = 8
TOKENS = B_ * S
T = TOKENS // N_CORES  # tokens per core
NUM_EXPERTS = 8
TOP_K = 2
SCALING = 16.0 / 64.0
R = 64  # total LoRA rank (8 experts x 8)
RB = R + 1  # + ones row (bias)

_BUILT = None


def _build():
    import concourse.bacc as bacc
    import concourse.mybir as mybir
    from concourse.bass import ts
    from concourse.tile import TileContext

    dt = mybir.dt
    f32 = dt.float32
    f32r = dt.float32r
    bf16 = dt.bfloat16
    P = 128
    DT = D // P          # 32 d-tiles
    TT = T // P          # 8 token tiles per core
    OTILE = 512
    NOT = O // OTILE     # 8 o-tiles
    HG = 512             # tokens per hT psum group

    nc = bacc.Bacc("TRN2", target_bir_lowering=False, debug=False)
    xT = nc.dram_tensor("xT", [D, T], f32r, kind="ExternalInput")
    wT = nc.dram_tensor("WT", [D, O], f32r, kind="ExternalInput")
    aT = nc.dram_tensor("AT", [D, R], f32r, kind="ExternalInput")
    bc = nc.dram_tensor("BC", [R, O], bf16, kind="ExternalInput")
    bias_d = nc.dram_tensor("BIAS", [1, O], f32, kind="ExternalInput")
    out = nc.dram_tensor("OUT", [T, O], f32, kind="ExternalOutput")

    with TileContext(nc) as tc:
        with (
            tc.tile_pool(name="resident", bufs=1) as res,
            tc.tile_pool(name="wpool", bufs=8) as wpool,
            tc.tile_pool(name="opool", bufs=10) as opool,
        ):
            x_sb = res.tile([P, DT, T], f32r, tag="x_sb")
            a_sb = res.tile([P, DT, R], f32r, tag="a_sb")
            b_sb = res.tile([P, O], bf16, tag="b_sb")
            h_sb = res.tile([P, T], bf16, tag="h_sb")
            bias_sb = res.tile([P, O], f32, tag="bias_sb")

            # Zero-pad the bf16 LoRA operands to 128 partitions so the LoRA
            # matmul contracts over a full 128 partitions (rows >= R are 0).
            nc.any.memzero(b_sb[:])
            nc.any.memzero(h_sb[:])

            # Main GEMM: for each o-tile keep all 8 token-tile PSUMs live and
            # stream W d-tile by d-tile (each W tile feeds 8 matmuls).
            # o-tile 0 also streams in x (paired with W per d-tile so the PE
            # can start within a few us); its LoRA term is applied post-hoc
            # once hT exists, so the PE never waits on the full x residency.
            with tc.tile_pool(name="psum", bufs=8, space="PSUM") as pp:
                for oti in range(NOT):
                    first = oti == 0
                    osl = slice(oti * OTILE, (oti + 1) * OTILE)
                    psums = [
                        pp.tile([P, OTILE], f32, tag="pout", name=f"pout_{oti}_{t}")
                        for t in range(TT)
                    ]
                    for dti in range(DT):
                        if first:
                            nc.sync.dma_start(
                                x_sb[:, dti, :], xT[dti * P:(dti + 1) * P, :]
                            )
                            nc.sync.dma_start(
                                a_sb[:, dti, :], aT[dti * P:(dti + 1) * P, :]
                            )
                        w_t = wpool.tile([P, OTILE], f32r, tag="w_t")
                        nc.sync.dma_start(
                            w_t[:], wT[dti * P:(dti + 1) * P, osl]
                        )
                        for t in range(TT):
                            nc.tensor.matmul(
                                psums[t][:],
                                lhsT=x_sb[:, dti, ts(t, P)],
                                rhs=w_t[:],
                                start=(dti == 0),
                                stop=(first and dti == DT - 1),
                            )
                    if first:
                        # Evict main+bias now (frees PSUM banks for hT);
                        # hold the SBUF tiles for the post-hoc LoRA add.
                        nc.sync.dma_start(b_sb[:R, :], bc[:])
                        nc.sync.dma_start(
                            bias_sb[:], bias_d[:].to_broadcast((P, O))
                        )
                        held = []
                        for t in range(TT):
                            o_t = opool.tile(
                                [P, OTILE], f32, tag="o_t", name=f"o0_{t}"
                            )
                            nc.vector.tensor_add(
                                out=o_t[:], in0=psums[t][:], in1=bias_sb[:, osl]
                            )
                            held.append(o_t)
                        # hT[r, tok] = sum_d A_all[r, d] * x[tok, d]
                        for g in range(T // HG):
                            ph = pp.tile([R, HG], f32, tag="pout", name=f"ph_{g}")
                            for dti in range(DT):
                                nc.tensor.matmul(
                                    ph[:],
                                    lhsT=a_sb[:, dti, :],
                                    rhs=x_sb[:, dti, g * HG:(g + 1) * HG],
                                    start=(dti == 0),
                                    stop=(dti == DT - 1),
                                )
                            nc.vector.tensor_copy(
                                out=h_sb[0:R, g * HG:(g + 1) * HG], in_=ph[:]
                            )
                        # Post-hoc LoRA for o-tile 0
                        for t in range(TT):
                            lp = pp.tile(
                                [P, OTILE], f32, tag="pout", name=f"lp_{t}"
                            )
                            nc.tensor.matmul(
                                lp[:],
                                lhsT=h_sb[:, ts(t, P)],
                                rhs=b_sb[:, osl],
                                start=True,
                                stop=True,
                            )
                            nc.vector.tensor_add(
                                out=held[t][:], in0=lp[:], in1=held[t][:]
                            )
                            nc.sync.dma_start(out[ts(t, P), osl], held[t][:])
                    else:
                        last = oti == NOT - 1
                        for t in range(TT):
                            # LoRA accumulated straight into the PSUM group
                            nc.tensor.matmul(
                                psums[t][:],
                                lhsT=h_sb[:, ts(t, P)],
                                rhs=b_sb[:, osl],
                                start=False,
                                stop=True,
                            )
                            o_t = opool.tile([P, OTILE], f32, tag="o_t")
                            nc.vector.tensor_add(
                                out=o_t[:], in0=psums[t][:], in1=bias_sb[:, osl]
                            )
                            # final o-tile: spread the tail stores over both
                            # HWDGE queues so the kernel tail drains faster
                            eng = nc.scalar if (last and t % 2 == 1) else nc.sync
                            eng.dma_start(out[ts(t, P), osl], o_t[:])

    nc.compile()
    return nc


def _get_nc():
    global _BUILT
    if _BUILT is None:
        _BUILT = _build()
    return _BUILT


def _host_prep(x, W, bias, A, B, expert_embed, router_w):
    x = np.asarray(x, dtype=np.float32)
    W = np.asarray(W, dtype=np.float32)
    bias = np.asarray(bias, dtype=np.float32)
    A = np.asarray(A, dtype=np.float32)
    B = np.asarray(B, dtype=np.float32)
    expert_embed = np.asarray(expert_embed, dtype=np.float32)
    router_w = np.asarray(router_w, dtype=np.float32)

    # Router (per-task, 8 scalars)
    logits = (expert_embed[0] @ router_w.T).astype(np.float32)
    e = np.exp(logits - logits.max())
    probs = (e / e.sum()).astype(np.float32)
    sel = np.argsort(-probs, kind="stable")[:TOP_K]
    gate = np.zeros(NUM_EXPERTS, np.float32)
    gate[sel] = probs[sel] * np.float32(SCALING)

    import ml_dtypes

    # Gate-scaled B, transposed to [r_total, O], cast bf16 for the device.
    Bc = (B.transpose(0, 2, 1) * gate[:, None, None]).reshape(R, O)
    BC = np.ascontiguousarray(Bc, dtype=ml_dtypes.bfloat16)
    BIAS = np.ascontiguousarray(bias.reshape(1, O), dtype=np.float32)
    AT = np.ascontiguousarray(A.reshape(R, D).T)
    WT = np.ascontiguousarray(W.T)

    xflat = x.reshape(TOKENS, D)
    in_maps = []
    for c in range(N_CORES):
        xt_shard = np.ascontiguousarray(xflat[c * T:(c + 1) * T, :].T)
        in_maps.append(
            {"xT": xt_shard, "WT": WT, "AT": AT, "BC": BC, "BIAS": BIAS}
        )
    return in_maps


def _execute(in_maps, trace=False, **kwargs):
    from concourse.bass_utils import run_bass_kernel_spmd

    nc = _get_nc()
    return run_bass_kernel_spmd(
        nc, in_maps, core_ids=list(range(N_CORES)), trace=trace, **kwargs
    )


def kernel(x, W, bias, A, B, expert_embed, router_w):
    in_maps = _host_prep(x, W, bias, A, B, expert_embed, router_w)
    res = _execute(in_maps, trace=False)
    out = np.concatenate([r["OUT"] for r in res.results], axis=0)
    return out.reshape(B_, S, O).astype(np.float32, copy=False)

